# revision 34
# baseline (speedup 1.0000x reference)
"""GQA (B=2, L=2048, D=2048, H=16, KVH=4, HD=128) on 8 Trainium2 NeuronCores.

Sharding: core c = (batch b = c//4, kv-group g = c%4). Each core computes its
group's 4 query heads + 1 KV head end-to-end and a partial output projection
(Wo in-dim slice); the host sums the 4 partials per batch (tensor-parallel
unshard) -- no on-device collectives.

Structure (measured 275 us on hardware vs 334 us for the first working
version; per-core PE roofline for this algorithm is ~196 us):
  A) Projections chunk-by-chunk (512 seq positions per chunk): QT/KT with RoPE
     fused into the PSUM eviction (ScalarE swap copies + DVE mult/adds, the
     attention scale folded into the Q rope tables); V computed as VT
     (stationary wv, streaming x -- no per-matmul LDWEIGHTS rebind) then
     turned into natural [seq, hd] tiles with PE transposes against an
     identity matrix. Phase A runs at ~96% of PE issue rate.
  B+C interleaved, chunk-major: for each chunk c, attention for all 4 heads
     (scores+exp software-pipelined one 2-j-tile group ahead of the attnV
     matmuls; [128,1024] PSUM score pairs so each ScalarE exp covers 2 tiles),
     with the previous chunk's output projection emitted between heads so its
     PE matmuls fill the pipeline while ScalarE works through the exps.
     Softmax row sums via an incremental DVE add chain over the probability
     tiles + two accumulating ones-matmul partition reductions per
     (head, chunk); reciprocal via exp(-ln) on ScalarE (DVE reciprocal is
     3.4us/tile on this build and the approx custom op doesn't codegen);
     causal masking by trimming matmuls on diagonal tiles + a gpsimd
     tri-mask multiply (exp runs full-width: the masked columns are never
     read downstream).
  Output bf16, packed [128, et*2048+...], DMA'd per (et, chunk) on the sync
  ring as soon as each tile is evicted; the final chunk's evictions are
  split DVE/ScalarE to shorten the drain tail.

All inputs host-packed into [128, N] layouts so every load is a handful of
large-line DMAs (16KB/partition) split across both HWDGE rings in
consumption order: wk/wv/k-rope first (the K/VT projections run first),
the first x chunk dripped in 256KB pieces across both rings, then
wq/q-rope/wo. First matmul issues ~12 us in (vs ~53 us for naive ordering).

PSUM budget (8 banks): score pairs 2x2 + attnV accumulator 2 + shared
row-sum/out-projection ring 2.
"""

import re
from contextlib import ExitStack

import ml_dtypes
import numpy as np

import concourse.bass as bass
import concourse.tile as tile
from concourse import mybir
from concourse.bass_utils import run_bass_kernel_spmd
from bass_rust import ScopedClock, VectorClock

dt = mybir.dt
BF16 = ml_dtypes.bfloat16

B, L, D = 2, 2048, 2048
H, KVH, HD = 16, 4, 128
G = H // KVH          # 4 query heads per kv head (= per core)
GD = G * HD           # 512: per-core q-head feature dim
THETA = 10000.0
SCALE = HD ** -0.5
NLT = L // 128        # 16 l-tiles
NDT = D // 128        # 16 d-tiles
NLC = L // 512        # 4 l-chunks


def _patch_tile_drain():
    """walrus in this container rejects multi-wait instructions on the SP
    queue; split the TileContext exit drain into one drain per proc."""
    def _drain_and_barrier_split(self, tick_clock, wait_clock):
        ticks = [int(s) for s in re.findall(r"\d+", str(tick_clock.global_clock))]
        for proc, t in enumerate(ticks):
            if t <= 0:
                continue
            vc = VectorClock()
            vc.require_at_least(proc, t)
            d = self.nc.sync.drain()
            wait_clock.add_sem_waits(d.ins, ScopedClock({None: vc}))
        self.nc.all_engine_barrier()
        assert self.sems is not None
        popped = self.nc._tile_sem_poison_stack.pop()
        assert popped is self._sem_poison
        self.nc.clear_and_free_semaphores(list(self.sems.allocated().values()))
        self.nc.all_engine_barrier()

    tile.TileContext._drain_and_barrier = _drain_and_barrier_split


def _split_multi_waits(nc):
    """This walrus build supports one sem-wait command per instruction; hoist
    excess waits onto same-engine NoOps inserted immediately before."""
    uid = 0
    for fn in nc.m.functions:
        for bb in fn.blocks:
            out = []
            for inst in bb.instructions:
                si = inst.sync_info
                if si is not None and si.on_wait and len(si.on_wait) > 1:
                    for w in si.on_wait[:-1]:
                        nop = mybir.InstNoOp(name=f"waitsplit-{uid}", ins=[], outs=[])
                        uid += 1
                        nop.engine = inst.engine
                        nop.sync_info = mybir.SyncInfo(on_wait=[w], on_update=[])
                        out.append(nop)
                    inst.sync_info = mybir.SyncInfo(
                        on_wait=[si.on_wait[-1]], on_update=si.on_update)
                out.append(inst)
            bb.instructions[:] = out


def _build_program():
    _patch_tile_drain()
    nc = bass.Bass("TRN2", target_bir_lowering=False, debug=False)

    xp = nc.dram_tensor("xp", [128, NLC * NDT * 512], dt.bfloat16, kind="ExternalInput").ap()
    wqp = nc.dram_tensor("wqp", [128, NDT * GD], dt.bfloat16, kind="ExternalInput").ap()
    wkp = nc.dram_tensor("wkp", [128, NDT * HD], dt.bfloat16, kind="ExternalInput").ap()
    wvp = nc.dram_tensor("wvp", [128, NDT * HD], dt.bfloat16, kind="ExternalInput").ap()
    wop = nc.dram_tensor("wop", [128, G * D], dt.bfloat16, kind="ExternalInput").ap()
    ropep = nc.dram_tensor("ropep", [128, 4 * L], dt.bfloat16, kind="ExternalInput").ap()
    # [tri | identity] constants, 128x128 each
    constp = nc.dram_tensor("constp", [128, 256], dt.bfloat16, kind="ExternalInput").ap()
    outp = nc.dram_tensor("outp", [128, NDT * L], dt.bfloat16, kind="ExternalOutput").ap()

    with tile.TileContext(nc) as tc:
        with ExitStack() as ctx:
            persist = ctx.enter_context(tc.tile_pool(name="persist", bufs=1))

            wq_sb = persist.tile([128, NDT * GD], dt.bfloat16, tag="wq", name="wq")
            wk_sb = persist.tile([128, NDT * HD], dt.bfloat16, tag="wk", name="wk")
            wv_sb = persist.tile([128, NDT * HD], dt.bfloat16, tag="wv", name="wv")
            wo_sb = persist.tile([128, G * D], dt.bfloat16, tag="wo", name="wo")
            rope_sb = persist.tile([128, 4 * L], dt.bfloat16, tag="rope", name="rope")
            const_sb = persist.tile([128, 256], dt.bfloat16, tag="const", name="const")
            ones_sb = persist.tile([128, 128], dt.bfloat16, tag="ones", name="ones")
            qt_sb = [persist.tile([HD, L], dt.bfloat16, tag=f"qt{h}", name=f"qt{h}") for h in range(G)]
            kt_sb = persist.tile([HD, L], dt.bfloat16, tag="kt", name="kt")
            # v chunk tiles: vc_sb[lc][:, k*128:(k+1)*128] = natural-V j-tile lc*4+k
            vc_sb = [persist.tile([128, 512], dt.bfloat16, tag=f"vc{lc}", name=f"vc{lc}") for lc in range(NLC)]
            ot_sb = [persist.tile([HD, L], dt.bfloat16, tag=f"ot{h}", name=f"ot{h}") for h in range(G)]

            cosq_sb = rope_sb[:, 0 * L:1 * L]
            sinq_sb = rope_sb[:, 1 * L:2 * L]
            cosk_sb = rope_sb[:, 2 * L:3 * L]
            sink_sb = rope_sb[:, 3 * L:4 * L]
            tri_sb = const_sb[:, 0:128]
            id_sb = const_sb[:, 128:256]

            def v_tile(jt):
                return vc_sb[jt // 4][:, (jt % 4) * 128:(jt % 4 + 1) * 128]

            # weights/tables on the scalar HWDGE ring in need-order: wk/wv
            # first (the K/VT projections run first), then wq BEFORE the rope
            # tables -- the K-eviction tolerates a late k-rope via the psA
            # ring, while the Q projections would stall the PE on a late wq
            nc.scalar.dma_start(out=wk_sb, in_=wkp)
            nc.scalar.dma_start(out=wv_sb, in_=wvp)
            for s in range(4):
                nc.scalar.dma_start(out=wq_sb[:, s * 4 * GD:(s + 1) * 4 * GD],
                                    in_=wqp[:, s * 4 * GD:(s + 1) * 4 * GD])
            nc.scalar.dma_start(out=rope_sb, in_=ropep)
            nc.scalar.dma_start(out=const_sb, in_=constp)
            nc.scalar.dma_start(out=wo_sb, in_=wop)
            nc.vector.memset(ones_sb, 1.0)

            # ---------------- Phase A: projections + rope ----------------
            with ExitStack() as ctxA, tc.spectator_scope("phaseA"):
                xpool = ctxA.enter_context(tc.tile_pool(name="xchunk", bufs=2))
                ropep_pool = ctxA.enter_context(tc.tile_pool(name="rope", bufs=4))
                vtep = ctxA.enter_context(tc.tile_pool(name="vte", bufs=2))
                psA = ctxA.enter_context(tc.tile_pool(name="psA", bufs=4, space="PSUM"))
                psVT = ctxA.enter_context(tc.tile_pool(name="psVT", bufs=2, space="PSUM"))
                psT = ctxA.enter_context(tc.tile_pool(name="psT", bufs=2, space="PSUM"))

                def rope_evict(ps, dst_slice, cos_t, sin_t, lc):
                    cs = cos_t[:, lc * 512:(lc + 1) * 512]
                    sn = sin_t[:, lc * 512:(lc + 1) * 512]
                    raw = ropep_pool.tile([128, 512], dt.bfloat16, tag="raw", name="raw")
                    swp = ropep_pool.tile([128, 512], dt.bfloat16, tag="swp", name="swp")
                    nc.scalar.copy(raw, ps)
                    nc.scalar.copy(swp[0:64, :], ps[64:128, :])
                    nc.scalar.copy(swp[64:128, :], ps[0:64, :])
                    t1 = ropep_pool.tile([128, 512], dt.bfloat16, tag="t1", name="t1")
                    t2 = ropep_pool.tile([128, 512], dt.bfloat16, tag="t2", name="t2")
                    nc.vector.tensor_tensor(t1, swp, sn, mybir.AluOpType.mult)
                    nc.vector.tensor_tensor(t2, raw, cs, mybir.AluOpType.mult)
                    nc.vector.tensor_tensor(dst_slice, t1, t2, mybir.AluOpType.add)

                vt_pending = None  # (vt_sbuf_tile, lc) awaiting PE transposes
                for lc in range(NLC):
                    xc = xpool.tile([128, NDT * 512], dt.bfloat16, tag="xc", name="xc")
                    if lc == 0:
                        # fine-grained drip on the sync ring so the K/VT
                        # groups can start as the first d-tiles land
                        for s in range(8):
                            sl = slice(s * 2 * 512, (s + 1) * 2 * 512)
                            nc.sync.dma_start(out=xc[:, sl], in_=xp[:, sl])
                    else:
                        nc.sync.dma_start(out=xc, in_=xp[:, lc * NDT * 512:(lc + 1) * NDT * 512])

                    # K and VT first: they only need the small wk/wv weights,
                    # so the PE starts well before the wq load completes
                    ps = psA.tile([128, 512], dt.float32, tag="psA", name="psA")
                    for i in range(NDT):
                        nc.tensor.matmul(ps, wk_sb[:, i * HD:(i + 1) * HD],
                                         xc[:, i * 512:(i + 1) * 512],
                                         start=(i == 0), stop=(i == NDT - 1))
                    rope_evict(ps, kt_sb[:, lc * 512:(lc + 1) * 512], cosk_sb, sink_sb, lc)

                    # VT = wv.T @ x chunk  [hd, 512]
                    pvt = psVT.tile([128, 512], dt.float32, tag="psVT", name="psVT")
                    for i in range(NDT):
                        nc.tensor.matmul(pvt, wv_sb[:, i * HD:(i + 1) * HD],
                                         xc[:, i * 512:(i + 1) * 512],
                                         start=(i == 0), stop=(i == NDT - 1))
                    vt_sb = vtep.tile([128, 512], dt.bfloat16, tag="vt", name="vt")
                    nc.vector.tensor_copy(vt_sb, pvt)

                    for ot in range(G):
                        ps = psA.tile([128, 512], dt.float32, tag="psA", name="psA")
                        for i in range(NDT):
                            nc.tensor.matmul(ps, wq_sb[:, i * GD + ot * 128:i * GD + (ot + 1) * 128],
                                             xc[:, i * 512:(i + 1) * 512],
                                             start=(i == 0), stop=(i == NDT - 1))
                        rope_evict(ps, qt_sb[ot][:, lc * 512:(lc + 1) * 512], cosq_sb, sinq_sb, lc)

                    # transpose previous chunk's VT into natural V tiles
                    # (deferred one chunk so the PE never waits on the evict)
                    def emit_transposes(vt_tile, vlc):
                        pt_ = psT.tile([128, 512], dt.bfloat16, tag="psT", name="psT")
                        for k in range(4):
                            nc.tensor.transpose(pt_[:, k * 128:(k + 1) * 128],
                                                vt_tile[:, k * 128:(k + 1) * 128], id_sb)
                        nc.vector.tensor_copy(vc_sb[vlc], pt_)

                    if vt_pending is not None:
                        emit_transposes(*vt_pending)
                    vt_pending = (vt_sb, lc)
                emit_transposes(*vt_pending)

            # ---------------- Phase B+C interleaved ----------------
            with ExitStack() as ctxBC, tc.spectator_scope("phaseBC"):
                psS = ctxBC.enter_context(tc.tile_pool(name="psS", bufs=2, space="PSUM"))
                psO = ctxBC.enter_context(tc.tile_pool(name="psO", bufs=2, space="PSUM"))
                # pr (softmax row-sum) and pw (out-proj) share one 2-bank pool
                psWR = ctxBC.enter_context(tc.tile_pool(name="psWR", bufs=2, space="PSUM"))
                ptp = ctxBC.enter_context(tc.tile_pool(name="pt", bufs=3))
                smp = ctxBC.enter_context(tc.tile_pool(name="sm", bufs=6))
                evp = ctxBC.enter_context(tc.tile_pool(name="ev", bufs=4))

                def emit_B(h, c, filler):
                    """attention for (head h, q-chunk c); scores+exp run one
                    2-j-tile group ahead of the attnV matmuls. `filler()` is
                    called between tile-groups to weave in out-projection
                    matmuls (PE work with no dependency on this group) so the
                    PE never drains while ScalarE works through the exps."""
                    qs = qt_sb[h][:, c * 512:(c + 1) * 512]
                    njt = 4 * (c + 1)
                    nbi = njt // 2
                    if c == 0:
                        # chunk 0 has no out-projection filler and tiny
                        # attention groups, so its po tiles rotate across BOTH
                        # PSUM rings (psWR is otherwise idle here) and pr goes
                        # to the score ring: all 4 head-groups stay in flight
                        po_pool, po_tag = (psO, "psO") if h < 2 else (psWR, "psWR")
                        po = po_pool.tile([128, 512], dt.float32, tag=po_tag, name="po")
                    else:
                        po = psO.tile([128, 512], dt.float32, tag="psO", name="psO")
                    # row-sum accumulator (DVE chain), halves reduced by two
                    # accumulating ones-matmuls at the end
                    acc = smp.tile([128, 1024], dt.bfloat16, tag="acc", name="acc")
                    pt_t = [None] * nbi

                    def offs(jt):
                        return (jt - 4 * c) * 128 if jt >= 4 * c else 0

                    def emit_scores_exp(bi):
                        jts = [2 * bi, 2 * bi + 1]
                        ps = psS.tile([128, 1024], dt.float32, tag="psS", name="psS")
                        for k, jt in enumerate(jts):
                            off = offs(jt)
                            nc.tensor.matmul(
                                ps[:, k * 512 + off:(k + 1) * 512],
                                kt_sb[:, jt * 128:(jt + 1) * 128],
                                qs[:, off:], start=True, stop=True)
                        pt = ptp.tile([128, 1024], dt.bfloat16, tag="pt", name="pt")
                        # one full-width exp also for diagonal pairs (the
                        # below-off columns hold unmasked junk no consumer
                        # reads); 128x128 diagonal-block masking on gpsimd
                        nc.scalar.activation(pt, ps, mybir.ActivationFunctionType.Exp)
                        if jts[1] >= 4 * c:
                            for k, jt in enumerate(jts):
                                off = offs(jt)
                                dd = pt[:, k * 512 + off:k * 512 + off + 128]
                                nc.gpsimd.tensor_tensor(dd, dd, tri_sb,
                                                        mybir.AluOpType.mult)
                        pt_t[bi] = pt

                    def emit_av(bi):
                        jts = [2 * bi, 2 * bi + 1]
                        pt = pt_t[bi]
                        for k, jt in enumerate(jts):
                            off = offs(jt)
                            pk = pt[:, k * 512 + off:(k + 1) * 512]
                            nc.tensor.matmul(po[:, off:], v_tile(jt), pk,
                                             start=(jt == 0), stop=(jt == njt - 1))
                        # incremental row-sum accumulation on DVE
                        diag = jts[1] >= 4 * c
                        if bi == 0:
                            if not diag:
                                nc.vector.tensor_copy(acc, pt)
                            else:  # only c==0: zero the masked column ranges
                                nc.vector.tensor_copy(acc[:, 0:512], pt[:, 0:512])
                                nc.vector.memset(acc[:, 512:640], 0.0)
                                nc.vector.tensor_copy(acc[:, 640:1024], pt[:, 640:1024])
                        elif not diag:
                            nc.vector.tensor_tensor(acc, acc, pt, mybir.AluOpType.add)
                        else:
                            for k, jt in enumerate(jts):
                                off = offs(jt)
                                sl = slice(k * 512 + off, (k + 1) * 512)
                                nc.vector.tensor_tensor(acc[:, sl], acc[:, sl], pt[:, sl],
                                                        mybir.AluOpType.add)
                        pt_t[bi] = None

                    emit_scores_exp(0)
                    for bi in range(nbi):
                        if bi + 1 < nbi:
                            emit_scores_exp(bi + 1)
                        emit_av(bi)
                        filler()

                    # partition-reduce the accumulator halves directly on the
                    # PE; reciprocal via exp(-ln) on ScalarE (DVE recip is
                    # slow on this build)
                    if c == 0:
                        pr = psS.tile([128, 512], dt.float32, tag="psS", name="pr")
                    else:
                        pr = psWR.tile([128, 512], dt.float32, tag="psWR", name="pr")
                    nc.tensor.matmul(pr, ones_sb, acc[:, 0:512], start=True, stop=False)
                    nc.tensor.matmul(pr, ones_sb, acc[:, 512:1024], start=False, stop=True)
                    lnr = smp.tile([128, 512], dt.float32, tag="lnr", name="lnr")
                    nc.scalar.activation(lnr, pr, mybir.ActivationFunctionType.Ln)
                    rcp = smp.tile([128, 512], dt.float32, tag="rcp", name="rcp")
                    nc.scalar.activation(rcp, lnr, mybir.ActivationFunctionType.Exp, scale=-1.0)
                    nc.vector.tensor_tensor(ot_sb[h][:, c * 512:(c + 1) * 512], po, rcp,
                                            mybir.AluOpType.mult)

                def emit_C_tile(lc, et, scalar_evict=False, use_psO=False):
                    """one out-projection row-tile for chunk lc"""
                    if use_psO:
                        # final chunk only: attention is done, so the psO ring
                        # is free -- using it doubles the out-proj pipeline
                        pw = psO.tile([128, 512], dt.float32, tag="psO", name="pw")
                    else:
                        pw = psWR.tile([128, 512], dt.float32, tag="psWR", name="pw")
                    for ot in range(G):
                        nc.tensor.matmul(pw, wo_sb[:, ot * D + et * 128:ot * D + (et + 1) * 128],
                                         ot_sb[ot][:, lc * 512:(lc + 1) * 512],
                                         start=(ot == 0), stop=(ot == G - 1))
                    ev = evp.tile([128, 512], dt.bfloat16, tag="ev", name="ev")
                    if scalar_evict:
                        nc.scalar.copy(ev, pw)
                    else:
                        nc.vector.tensor_copy(ev, pw)
                    nc.sync.dma_start(
                        out=outp[:, et * L + lc * 512:et * L + (lc + 1) * 512], in_=ev)

                def no_filler():
                    pass

                for c in range(NLC):
                    for h in range(G):
                        emit_B(h, c, no_filler)
                        # the previous chunk's output projection is spread
                        # across this chunk's heads (4 row-tiles per head)
                        if c > 0:
                            for et in range(h * 4, (h + 1) * 4):
                                emit_C_tile(c - 1, et)
                # final chunk: attention is done, so ScalarE helps with half
                # the evictions and the psO ring doubles the PSUM pipeline,
                # shortening the drain tail
                for et in range(NDT):
                    emit_C_tile(NLC - 1, et, scalar_evict=(et % 2 == 1),
                                use_psO=(et % 2 == 1))
    _split_multi_waits(nc)
    return nc


_PROG = None


def _rope_tables():
    inv_freq = 1.0 / (THETA ** (np.arange(0, HD, 2, dtype=np.float32) / HD))
    t = np.arange(L, dtype=np.float32)
    freqs = np.outer(t, inv_freq)
    emb = np.concatenate([freqs, freqs], axis=-1)      # [L, HD]
    cos = np.cos(emb).T.copy()                         # [HD, L]
    sin = np.sin(emb).T.copy()
    sin_eff = sin.copy()
    sin_eff[:64] = -sin_eff[:64]                       # dest-indexed rotate_half sign
    return cos, sin_eff


def _prepare_in_maps(x, Wq, Wk, Wv, Wo):
    cos, sin_eff = _rope_tables()
    bfc = lambda a: np.ascontiguousarray(a).astype(BF16)
    ropep = bfc(np.concatenate([cos * SCALE, sin_eff * SCALE, cos, sin_eff], axis=1))
    tri = np.tril(np.ones((128, 128), dtype=np.float32)).T  # 1 where pj <= fq
    constp = bfc(np.concatenate([tri, np.eye(128, dtype=np.float32)], axis=1))

    x, Wq, Wk, Wv, Wo = (np.asarray(a) for a in (x, Wq, Wk, Wv, Wo))
    xpb = []
    for b in range(B):
        xT = x[b].T                                            # [D, L]
        xpb.append(bfc(xT.reshape(NDT, 128, NLC, 512).transpose(1, 2, 0, 3)
                       .reshape(128, NLC * NDT * 512)))

    in_maps = []
    for c in range(8):
        b, g = c // 4, c % 4
        wqT = Wq[g * GD:(g + 1) * GD, :].T                     # [D, GD]
        wkT = Wk[g * HD:(g + 1) * HD, :].T                     # [D, HD]
        wvT = Wv[g * HD:(g + 1) * HD, :].T
        woT = Wo[:, g * GD:(g + 1) * GD].T                     # [GD, D]
        in_maps.append({
            "xp": xpb[b],
            "wqp": bfc(wqT.reshape(NDT, 128, GD).transpose(1, 0, 2).reshape(128, NDT * GD)),
            "wkp": bfc(wkT.reshape(NDT, 128, HD).transpose(1, 0, 2).reshape(128, NDT * HD)),
            "wvp": bfc(wvT.reshape(NDT, 128, HD).transpose(1, 0, 2).reshape(128, NDT * HD)),
            "wop": bfc(woT.reshape(G, 128, D).transpose(1, 0, 2).reshape(128, G * D)),
            "ropep": ropep,
            "constp": constp,
        })
    return in_maps


def _run(in_maps, **kwargs):
    global _PROG
    if _PROG is None:
        _PROG = _build_program()
    return run_bass_kernel_spmd(_PROG, in_maps, list(range(8)), **kwargs)


def _gather(res):
    out = np.zeros((B, L, D), dtype=np.float32)
    for c in range(8):
        b = c // 4
        outp = res.results[c]["outp"]                          # [128, 16*2048] bf16
        outT = outp.reshape(128, NDT, L).transpose(1, 0, 2).reshape(D, L)
        out[b] += outT.T.astype(np.float32)
    return out


def kernel(x, Wq, Wk, Wv, Wo):
    return _gather(_run(_prepare_in_maps(x, Wq, Wk, Wv, Wo)))


# revision 36
# speedup vs baseline: 1.0477x; 1.0477x over previous
"""GQA (B=2, L=2048, D=2048, H=16, KVH=4, HD=128) on 8 Trainium2 NeuronCores.

Sharding: core c = (batch b = c//4, kv-group g = c%4). Each core computes its
group's 4 query heads + 1 KV head end-to-end and a partial output projection
(Wo in-dim slice); the host sums the 4 partials per batch (tensor-parallel
unshard) -- no on-device collectives.

Structure (measured 275 us on hardware vs 334 us for the first working
version; per-core PE roofline for this algorithm is ~196 us):
  A) Projections chunk-by-chunk (512 seq positions per chunk): QT/KT with RoPE
     fused into the PSUM eviction (ScalarE swap copies + DVE mult/adds, the
     attention scale folded into the Q rope tables); V computed as VT
     (stationary wv, streaming x -- no per-matmul LDWEIGHTS rebind) then
     turned into natural [seq, hd] tiles with PE transposes against an
     identity matrix. Phase A runs at ~96% of PE issue rate.
  B+C interleaved, chunk-major: for each chunk c, attention for all 4 heads
     (scores+exp software-pipelined one 2-j-tile group ahead of the attnV
     matmuls; [128,1024] PSUM score pairs so each ScalarE exp covers 2 tiles),
     with the previous chunk's output projection emitted between heads so its
     PE matmuls fill the pipeline while ScalarE works through the exps.
     Softmax row sums via an incremental DVE add chain over the probability
     tiles + two accumulating ones-matmul partition reductions per
     (head, chunk); reciprocal via exp(-ln) on ScalarE (DVE reciprocal is
     3.4us/tile on this build and the approx custom op doesn't codegen);
     causal masking by trimming matmuls on diagonal tiles + a gpsimd
     tri-mask multiply (exp runs full-width: the masked columns are never
     read downstream).
  Output bf16, packed [128, et*2048+...], DMA'd per (et, chunk) on the sync
  ring as soon as each tile is evicted; the final chunk's evictions are
  split DVE/ScalarE to shorten the drain tail.

All inputs host-packed into [128, N] layouts so every load is a handful of
large-line DMAs (16KB/partition) split across both HWDGE rings in
consumption order: wk/wv/k-rope first (the K/VT projections run first),
the first x chunk dripped in 256KB pieces across both rings, then
wq/q-rope/wo. First matmul issues ~12 us in (vs ~53 us for naive ordering).

PSUM budget (8 banks): score pairs 2x2 + attnV accumulator 2 + shared
row-sum/out-projection ring 2.
"""

import re
from contextlib import ExitStack

import ml_dtypes
import numpy as np

import concourse.bass as bass
import concourse.tile as tile
from concourse import mybir
from concourse.bass_utils import run_bass_kernel_spmd
from bass_rust import ScopedClock, VectorClock

dt = mybir.dt
BF16 = ml_dtypes.bfloat16

B, L, D = 2, 2048, 2048
H, KVH, HD = 16, 4, 128
G = H // KVH          # 4 query heads per kv head (= per core)
GD = G * HD           # 512: per-core q-head feature dim
THETA = 10000.0
SCALE = HD ** -0.5
NLT = L // 128        # 16 l-tiles
NDT = D // 128        # 16 d-tiles
NLC = L // 512        # 4 l-chunks


def _patch_tile_drain():
    """walrus in this container rejects multi-wait instructions on the SP
    queue; split the TileContext exit drain into one drain per proc."""
    def _drain_and_barrier_split(self, tick_clock, wait_clock):
        ticks = [int(s) for s in re.findall(r"\d+", str(tick_clock.global_clock))]
        for proc, t in enumerate(ticks):
            if t <= 0:
                continue
            vc = VectorClock()
            vc.require_at_least(proc, t)
            d = self.nc.sync.drain()
            wait_clock.add_sem_waits(d.ins, ScopedClock({None: vc}))
        self.nc.all_engine_barrier()
        assert self.sems is not None
        popped = self.nc._tile_sem_poison_stack.pop()
        assert popped is self._sem_poison
        self.nc.clear_and_free_semaphores(list(self.sems.allocated().values()))
        self.nc.all_engine_barrier()

    tile.TileContext._drain_and_barrier = _drain_and_barrier_split


def _split_multi_waits(nc):
    """This walrus build supports one sem-wait command per instruction; hoist
    excess waits onto same-engine NoOps inserted immediately before."""
    uid = 0
    for fn in nc.m.functions:
        for bb in fn.blocks:
            out = []
            for inst in bb.instructions:
                si = inst.sync_info
                if si is not None and si.on_wait and len(si.on_wait) > 1:
                    for w in si.on_wait[:-1]:
                        nop = mybir.InstNoOp(name=f"waitsplit-{uid}", ins=[], outs=[])
                        uid += 1
                        nop.engine = inst.engine
                        nop.sync_info = mybir.SyncInfo(on_wait=[w], on_update=[])
                        out.append(nop)
                    inst.sync_info = mybir.SyncInfo(
                        on_wait=[si.on_wait[-1]], on_update=si.on_update)
                out.append(inst)
            bb.instructions[:] = out


def _build_program():
    _patch_tile_drain()
    nc = bass.Bass("TRN2", target_bir_lowering=False, debug=False)

    xp = nc.dram_tensor("xp", [128, NLC * NDT * 512], dt.bfloat16, kind="ExternalInput").ap()
    wqp = nc.dram_tensor("wqp", [128, NDT * GD], dt.bfloat16, kind="ExternalInput").ap()
    wkp = nc.dram_tensor("wkp", [128, NDT * HD], dt.bfloat16, kind="ExternalInput").ap()
    wvp = nc.dram_tensor("wvp", [128, NDT * HD], dt.bfloat16, kind="ExternalInput").ap()
    wop = nc.dram_tensor("wop", [128, G * D], dt.bfloat16, kind="ExternalInput").ap()
    ropep = nc.dram_tensor("ropep", [128, 4 * L], dt.bfloat16, kind="ExternalInput").ap()
    # [tri | identity] constants, 128x128 each
    constp = nc.dram_tensor("constp", [128, 256], dt.bfloat16, kind="ExternalInput").ap()
    outp = nc.dram_tensor("outp", [128, NDT * L], dt.bfloat16, kind="ExternalOutput").ap()

    with tile.TileContext(nc) as tc:
        with ExitStack() as ctx:
            persist = ctx.enter_context(tc.tile_pool(name="persist", bufs=1))

            wq_sb = persist.tile([128, NDT * GD], dt.bfloat16, tag="wq", name="wq")
            wk_sb = persist.tile([128, NDT * HD], dt.bfloat16, tag="wk", name="wk")
            wv_sb = persist.tile([128, NDT * HD], dt.bfloat16, tag="wv", name="wv")
            wo_sb = persist.tile([128, G * D], dt.bfloat16, tag="wo", name="wo")
            rope_sb = persist.tile([128, 4 * L], dt.bfloat16, tag="rope", name="rope")
            const_sb = persist.tile([128, 256], dt.bfloat16, tag="const", name="const")
            ones_sb = persist.tile([128, 128], dt.bfloat16, tag="ones", name="ones")
            qt_sb = [persist.tile([HD, L], dt.bfloat16, tag=f"qt{h}", name=f"qt{h}") for h in range(G)]
            kt_sb = persist.tile([HD, L], dt.bfloat16, tag="kt", name="kt")
            # v chunk tiles: vc_sb[lc][:, k*128:(k+1)*128] = natural-V j-tile lc*4+k
            vc_sb = [persist.tile([128, 512], dt.bfloat16, tag=f"vc{lc}", name=f"vc{lc}") for lc in range(NLC)]
            ot_sb = [persist.tile([HD, L], dt.bfloat16, tag=f"ot{h}", name=f"ot{h}") for h in range(G)]

            cosq_sb = rope_sb[:, 0 * L:1 * L]
            sinq_sb = rope_sb[:, 1 * L:2 * L]
            cosk_sb = rope_sb[:, 2 * L:3 * L]
            sink_sb = rope_sb[:, 3 * L:4 * L]
            tri_sb = const_sb[:, 0:128]
            id_sb = const_sb[:, 128:256]

            def v_tile(jt):
                return vc_sb[jt // 4][:, (jt % 4) * 128:(jt % 4 + 1) * 128]

            # weights/tables on the scalar HWDGE ring in need-order: wk/wv
            # first (the K/VT projections run first), then wq BEFORE the rope
            # tables -- the K-eviction tolerates a late k-rope via the psA
            # ring, while the Q projections would stall the PE on a late wq
            nc.scalar.dma_start(out=wk_sb, in_=wkp)
            nc.scalar.dma_start(out=wv_sb, in_=wvp)
            for s in range(4):
                nc.scalar.dma_start(out=wq_sb[:, s * 4 * GD:(s + 1) * 4 * GD],
                                    in_=wqp[:, s * 4 * GD:(s + 1) * 4 * GD])
            nc.scalar.dma_start(out=rope_sb, in_=ropep)
            nc.scalar.dma_start(out=const_sb, in_=constp)
            nc.scalar.dma_start(out=wo_sb, in_=wop)
            nc.vector.memset(ones_sb, 1.0)

            # ---------------- Phase A: projections + rope ----------------
            with ExitStack() as ctxA, tc.spectator_scope("phaseA"):
                xpool = ctxA.enter_context(tc.tile_pool(name="xchunk", bufs=2))
                ropep_pool = ctxA.enter_context(tc.tile_pool(name="rope", bufs=4))
                vtep = ctxA.enter_context(tc.tile_pool(name="vte", bufs=2))
                psA = ctxA.enter_context(tc.tile_pool(name="psA", bufs=4, space="PSUM"))
                psVT = ctxA.enter_context(tc.tile_pool(name="psVT", bufs=2, space="PSUM"))
                psT = ctxA.enter_context(tc.tile_pool(name="psT", bufs=2, space="PSUM"))

                def rope_evict(ps, dst_slice, cos_t, sin_t, lc):
                    cs = cos_t[:, lc * 512:(lc + 1) * 512]
                    sn = sin_t[:, lc * 512:(lc + 1) * 512]
                    raw = ropep_pool.tile([128, 512], dt.bfloat16, tag="raw", name="raw")
                    swp = ropep_pool.tile([128, 512], dt.bfloat16, tag="swp", name="swp")
                    nc.scalar.copy(raw, ps)
                    nc.scalar.copy(swp[0:64, :], ps[64:128, :])
                    nc.scalar.copy(swp[64:128, :], ps[0:64, :])
                    t1 = ropep_pool.tile([128, 512], dt.bfloat16, tag="t1", name="t1")
                    t2 = ropep_pool.tile([128, 512], dt.bfloat16, tag="t2", name="t2")
                    nc.vector.tensor_tensor(t1, swp, sn, mybir.AluOpType.mult)
                    nc.vector.tensor_tensor(t2, raw, cs, mybir.AluOpType.mult)
                    nc.vector.tensor_tensor(dst_slice, t1, t2, mybir.AluOpType.add)

                vt_pending = None  # (vt_sbuf_tile, lc) awaiting PE transposes
                for lc in range(NLC):
                    xc = xpool.tile([128, NDT * 512], dt.bfloat16, tag="xc", name="xc")
                    if lc == 0:
                        # fine-grained drip on the sync ring so the K/VT
                        # groups can start as the first d-tiles land
                        for s in range(8):
                            sl = slice(s * 2 * 512, (s + 1) * 2 * 512)
                            nc.sync.dma_start(out=xc[:, sl], in_=xp[:, sl])
                    else:
                        nc.sync.dma_start(out=xc, in_=xp[:, lc * NDT * 512:(lc + 1) * NDT * 512])

                    # K and VT first: they only need the small wk/wv weights,
                    # so the PE starts well before the wq load completes
                    ps = psA.tile([128, 512], dt.float32, tag="psA", name="psA")
                    for i in range(NDT):
                        nc.tensor.matmul(ps, wk_sb[:, i * HD:(i + 1) * HD],
                                         xc[:, i * 512:(i + 1) * 512],
                                         start=(i == 0), stop=(i == NDT - 1))
                    rope_evict(ps, kt_sb[:, lc * 512:(lc + 1) * 512], cosk_sb, sink_sb, lc)

                    # VT = wv.T @ x chunk  [hd, 512]
                    pvt = psVT.tile([128, 512], dt.float32, tag="psVT", name="psVT")
                    for i in range(NDT):
                        nc.tensor.matmul(pvt, wv_sb[:, i * HD:(i + 1) * HD],
                                         xc[:, i * 512:(i + 1) * 512],
                                         start=(i == 0), stop=(i == NDT - 1))
                    vt_sb = vtep.tile([128, 512], dt.bfloat16, tag="vt", name="vt")
                    nc.vector.tensor_copy(vt_sb, pvt)

                    for ot in range(G):
                        ps = psA.tile([128, 512], dt.float32, tag="psA", name="psA")
                        for i in range(NDT):
                            nc.tensor.matmul(ps, wq_sb[:, i * GD + ot * 128:i * GD + (ot + 1) * 128],
                                             xc[:, i * 512:(i + 1) * 512],
                                             start=(i == 0), stop=(i == NDT - 1))
                        rope_evict(ps, qt_sb[ot][:, lc * 512:(lc + 1) * 512], cosq_sb, sinq_sb, lc)

                    # transpose previous chunk's VT into natural V tiles
                    # (deferred one chunk so the PE never waits on the evict)
                    def emit_transposes(vt_tile, vlc):
                        pt_ = psT.tile([128, 512], dt.bfloat16, tag="psT", name="psT")
                        for k in range(4):
                            nc.tensor.transpose(pt_[:, k * 128:(k + 1) * 128],
                                                vt_tile[:, k * 128:(k + 1) * 128], id_sb)
                        nc.vector.tensor_copy(vc_sb[vlc], pt_)

                    if vt_pending is not None:
                        emit_transposes(*vt_pending)
                    vt_pending = (vt_sb, lc)
                emit_transposes(*vt_pending)

            # ---------------- Phase B+C interleaved ----------------
            with ExitStack() as ctxBC, tc.spectator_scope("phaseBC"):
                psS = ctxBC.enter_context(tc.tile_pool(name="psS", bufs=2, space="PSUM"))
                psO = ctxBC.enter_context(tc.tile_pool(name="psO", bufs=2, space="PSUM"))
                # pr (softmax row-sum) and pw (out-proj) share one 2-bank pool
                psWR = ctxBC.enter_context(tc.tile_pool(name="psWR", bufs=2, space="PSUM"))
                ptp = ctxBC.enter_context(tc.tile_pool(name="pt", bufs=3))
                smp = ctxBC.enter_context(tc.tile_pool(name="sm", bufs=6))
                evp = ctxBC.enter_context(tc.tile_pool(name="ev", bufs=4))

                def emit_B(h, c, filler):
                    """attention for (head h, q-chunk c); scores+exp run one
                    2-j-tile group ahead of the attnV matmuls. `filler()` is
                    called between tile-groups to weave in out-projection
                    matmuls (PE work with no dependency on this group) so the
                    PE never drains while ScalarE works through the exps."""
                    qs = qt_sb[h][:, c * 512:(c + 1) * 512]
                    njt = 4 * (c + 1)
                    nbi = njt // 2
                    po = psO.tile([128, 512], dt.float32, tag="psO", name="psO")
                    # row-sum accumulator (DVE chain), halves reduced by two
                    # accumulating ones-matmuls at the end
                    acc = smp.tile([128, 1024], dt.bfloat16, tag="acc", name="acc")
                    pt_t = [None] * nbi

                    def offs(jt):
                        return (jt - 4 * c) * 128 if jt >= 4 * c else 0

                    def emit_scores_exp(bi):
                        jts = [2 * bi, 2 * bi + 1]
                        ps = psS.tile([128, 1024], dt.float32, tag="psS", name="psS")
                        for k, jt in enumerate(jts):
                            off = offs(jt)
                            nc.tensor.matmul(
                                ps[:, k * 512 + off:(k + 1) * 512],
                                kt_sb[:, jt * 128:(jt + 1) * 128],
                                qs[:, off:], start=True, stop=True)
                        pt = ptp.tile([128, 1024], dt.bfloat16, tag="pt", name="pt")
                        # one full-width exp also for diagonal pairs (the
                        # below-off columns hold unmasked junk no consumer
                        # reads); 128x128 diagonal-block masking on gpsimd
                        nc.scalar.activation(pt, ps, mybir.ActivationFunctionType.Exp)
                        if jts[1] >= 4 * c:
                            for k, jt in enumerate(jts):
                                off = offs(jt)
                                dd = pt[:, k * 512 + off:k * 512 + off + 128]
                                nc.gpsimd.tensor_tensor(dd, dd, tri_sb,
                                                        mybir.AluOpType.mult)
                        pt_t[bi] = pt

                    def emit_av(bi):
                        jts = [2 * bi, 2 * bi + 1]
                        pt = pt_t[bi]
                        for k, jt in enumerate(jts):
                            off = offs(jt)
                            pk = pt[:, k * 512 + off:(k + 1) * 512]
                            nc.tensor.matmul(po[:, off:], v_tile(jt), pk,
                                             start=(jt == 0), stop=(jt == njt - 1))
                        # incremental row-sum accumulation on DVE
                        diag = jts[1] >= 4 * c
                        if bi == 0:
                            if not diag:
                                nc.vector.tensor_copy(acc, pt)
                            else:  # only c==0: zero the masked column ranges
                                nc.vector.tensor_copy(acc[:, 0:512], pt[:, 0:512])
                                nc.vector.memset(acc[:, 512:640], 0.0)
                                nc.vector.tensor_copy(acc[:, 640:1024], pt[:, 640:1024])
                        elif not diag:
                            nc.vector.tensor_tensor(acc, acc, pt, mybir.AluOpType.add)
                        else:
                            for k, jt in enumerate(jts):
                                off = offs(jt)
                                sl = slice(k * 512 + off, (k + 1) * 512)
                                nc.vector.tensor_tensor(acc[:, sl], acc[:, sl], pt[:, sl],
                                                        mybir.AluOpType.add)
                        pt_t[bi] = None

                    emit_scores_exp(0)
                    for bi in range(nbi):
                        if bi + 1 < nbi:
                            emit_scores_exp(bi + 1)
                        emit_av(bi)
                        filler()

                    # partition-reduce the accumulator halves directly on the
                    # PE; reciprocal via exp(-ln) on ScalarE (DVE recip is
                    # slow on this build)
                    pr = psWR.tile([128, 512], dt.float32, tag="psWR", name="pr")
                    nc.tensor.matmul(pr, ones_sb, acc[:, 0:512], start=True, stop=False)
                    nc.tensor.matmul(pr, ones_sb, acc[:, 512:1024], start=False, stop=True)
                    lnr = smp.tile([128, 512], dt.float32, tag="lnr", name="lnr")
                    nc.scalar.activation(lnr, pr, mybir.ActivationFunctionType.Ln)
                    rcp = smp.tile([128, 512], dt.float32, tag="rcp", name="rcp")
                    nc.scalar.activation(rcp, lnr, mybir.ActivationFunctionType.Exp, scale=-1.0)
                    nc.vector.tensor_tensor(ot_sb[h][:, c * 512:(c + 1) * 512], po, rcp,
                                            mybir.AluOpType.mult)

                def emit_C_tile(lc, et, scalar_evict=False, use_psO=False):
                    """one out-projection row-tile for chunk lc"""
                    if use_psO:
                        # final chunk only: attention is done, so the psO ring
                        # is free -- using it doubles the out-proj pipeline
                        pw = psO.tile([128, 512], dt.float32, tag="psO", name="pw")
                    else:
                        pw = psWR.tile([128, 512], dt.float32, tag="psWR", name="pw")
                    for ot in range(G):
                        nc.tensor.matmul(pw, wo_sb[:, ot * D + et * 128:ot * D + (et + 1) * 128],
                                         ot_sb[ot][:, lc * 512:(lc + 1) * 512],
                                         start=(ot == 0), stop=(ot == G - 1))
                    ev = evp.tile([128, 512], dt.bfloat16, tag="ev", name="ev")
                    if scalar_evict:
                        nc.scalar.copy(ev, pw)
                    else:
                        nc.vector.tensor_copy(ev, pw)
                    nc.sync.dma_start(
                        out=outp[:, et * L + lc * 512:et * L + (lc + 1) * 512], in_=ev)

                def no_filler():
                    pass

                for c in range(NLC):
                    for h in range(G):
                        emit_B(h, c, no_filler)
                        # the previous chunk's output projection is spread
                        # across this chunk's heads (4 row-tiles per head)
                        if c > 0:
                            for et in range(h * 4, (h + 1) * 4):
                                emit_C_tile(c - 1, et)
                # final chunk: attention is done, so ScalarE helps with half
                # the evictions and the psO ring doubles the PSUM pipeline,
                # shortening the drain tail
                for et in range(NDT):
                    emit_C_tile(NLC - 1, et, scalar_evict=(et % 2 == 1),
                                use_psO=(et % 2 == 1))
    _split_multi_waits(nc)
    return nc


_PROG = None


def _rope_tables():
    inv_freq = 1.0 / (THETA ** (np.arange(0, HD, 2, dtype=np.float32) / HD))
    t = np.arange(L, dtype=np.float32)
    freqs = np.outer(t, inv_freq)
    emb = np.concatenate([freqs, freqs], axis=-1)      # [L, HD]
    cos = np.cos(emb).T.copy()                         # [HD, L]
    sin = np.sin(emb).T.copy()
    sin_eff = sin.copy()
    sin_eff[:64] = -sin_eff[:64]                       # dest-indexed rotate_half sign
    return cos, sin_eff


def _prepare_in_maps(x, Wq, Wk, Wv, Wo):
    cos, sin_eff = _rope_tables()
    bfc = lambda a: np.ascontiguousarray(a).astype(BF16)
    ropep = bfc(np.concatenate([cos * SCALE, sin_eff * SCALE, cos, sin_eff], axis=1))
    tri = np.tril(np.ones((128, 128), dtype=np.float32)).T  # 1 where pj <= fq
    constp = bfc(np.concatenate([tri, np.eye(128, dtype=np.float32)], axis=1))

    x, Wq, Wk, Wv, Wo = (np.asarray(a) for a in (x, Wq, Wk, Wv, Wo))
    xpb = []
    for b in range(B):
        xT = x[b].T                                            # [D, L]
        xpb.append(bfc(xT.reshape(NDT, 128, NLC, 512).transpose(1, 2, 0, 3)
                       .reshape(128, NLC * NDT * 512)))

    in_maps = []
    for c in range(8):
        b, g = c // 4, c % 4
        wqT = Wq[g * GD:(g + 1) * GD, :].T                     # [D, GD]
        wkT = Wk[g * HD:(g + 1) * HD, :].T                     # [D, HD]
        wvT = Wv[g * HD:(g + 1) * HD, :].T
        woT = Wo[:, g * GD:(g + 1) * GD].T                     # [GD, D]
        in_maps.append({
            "xp": xpb[b],
            "wqp": bfc(wqT.reshape(NDT, 128, GD).transpose(1, 0, 2).reshape(128, NDT * GD)),
            "wkp": bfc(wkT.reshape(NDT, 128, HD).transpose(1, 0, 2).reshape(128, NDT * HD)),
            "wvp": bfc(wvT.reshape(NDT, 128, HD).transpose(1, 0, 2).reshape(128, NDT * HD)),
            "wop": bfc(woT.reshape(G, 128, D).transpose(1, 0, 2).reshape(128, G * D)),
            "ropep": ropep,
            "constp": constp,
        })
    return in_maps


def _run(in_maps, **kwargs):
    global _PROG
    if _PROG is None:
        _PROG = _build_program()
    return run_bass_kernel_spmd(_PROG, in_maps, list(range(8)), **kwargs)


def _gather(res):
    out = np.zeros((B, L, D), dtype=np.float32)
    for c in range(8):
        b = c // 4
        outp = res.results[c]["outp"]                          # [128, 16*2048] bf16
        outT = outp.reshape(128, NDT, L).transpose(1, 0, 2).reshape(D, L)
        out[b] += outT.T.astype(np.float32)
    return out


def kernel(x, Wq, Wk, Wv, Wo):
    return _gather(_run(_prepare_in_maps(x, Wq, Wk, Wv, Wo)))
